# revision 49
# baseline (speedup 1.0000x reference)
"""Multi-head attention (B=4, T=S=2048, E=1024, H=16, D=64) on 8 TRN2 NeuronCores.

Sharding: core c handles batch b=c//2 and head-group g=c%2 (8 of 16 heads).
Each core computes its 8 heads' attention plus the matching column-slice of
the output projection, producing a partial [T, E] f32 output. Host sums the
two partials per batch and adds bo.

On-chip dataflow (all matmuls bf16 with fp32 PSUM accumulation):
  qT[d,t] = WqT.T @ queryT       (d-major projections, per 128-dim head pair)
  kT[d,t] likewise; v[s,d] natural via value.T as the stationary operand
  S.T[s,t] = kT_h.T @ qT_h       (two heads row-packed in the 128-row PE array)
  expS.T   = exp(S.T * 1/8)      (ScalarE, PSUM -> SBUF bf16)
  O[t,d]   = expS.T.T @ v_h      (exp tile stationary, v moving: charges 64
                                  cycles/matmul instead of 512 -> PV at its
                                  cost-model floor; denominators via 1-wide
                                  matmuls against a ones column)
  Onorm    = (O * 1/den).T       (DVE per-partition scalar mul, then an
                                  SBUF->SBUF DMA-transpose back to [d, t])
  partial  = Onorm.T @ WoSlice   (accumulate over the core's 4 head pairs)

Emission is software-pipelined: stage s=(pair, t-quarter); each stage's 16
score-tile slots interleave the previous stage's PV at 2 PV-slots per score
slot (normalize runs mid-stage so the single PV-accumulator PSUM bank is
clear well before reuse) plus spread-out projection / v-projection /
out-projection work, keeping both ScalarE (exp) and PE continuously fed.
"""

from contextlib import ExitStack

import numpy as np
import ml_dtypes

B, T, S, E = 4, 2048, 2048, 1024
H, D = 16, 64
DC = 512          # dims per core (8 heads x 64)
NP = 4            # head pairs per core
NS = S // 128     # 16 s-tiles
NQ = 4            # t-quarters of 512

_BF16 = ml_dtypes.bfloat16

_cached = None


def _build(repeats=1):
    import concourse.bass as bass
    import concourse.mybir as mybir
    import concourse.tile as tile
    from concourse import bacc

    f32 = mybir.dt.float32
    bf16 = mybir.dt.bfloat16
    AF = mybir.ActivationFunctionType

    nc = bacc.Bacc("TRN2", target_bir_lowering=False)

    qT_d = nc.dram_tensor("qT", [E, T], bf16, kind="ExternalInput")
    kT_d = nc.dram_tensor("kT", [E, S], bf16, kind="ExternalInput")
    vT_d = nc.dram_tensor("vT", [E, S], bf16, kind="ExternalInput")
    WqT_d = nc.dram_tensor("WqT", [E, DC], bf16, kind="ExternalInput")
    WkT_d = nc.dram_tensor("WkT", [E, DC], bf16, kind="ExternalInput")
    WvT_d = nc.dram_tensor("WvT", [E, DC], bf16, kind="ExternalInput")
    WoS_d = nc.dram_tensor("WoS", [DC, E], bf16, kind="ExternalInput")
    bq_d = nc.dram_tensor("bq", [128, NP], f32, kind="ExternalInput")
    bk_d = nc.dram_tensor("bk", [128, NP], f32, kind="ExternalInput")
    bv_d = nc.dram_tensor("bv", [1, DC], f32, kind="ExternalInput")
    out_d = nc.dram_tensor("out", [T, E], f32, kind="ExternalOutput")

    with tile.TileContext(nc) as tc, ExitStack() as ctx:
        persist = ctx.enter_context(tc.tile_pool(name="persist", bufs=1))
        psc = ctx.enter_context(tc.tile_pool(name="psc", bufs=2, space="PSUM"))
        pacc = ctx.enter_context(tc.tile_pool(name="pacc", bufs=1, space="PSUM"))
        pden = ctx.enter_context(tc.tile_pool(name="pden", bufs=1, space="PSUM"))
        pmx = ctx.enter_context(tc.tile_pool(name="pmx", bufs=2, space="PSUM"))
        expool = ctx.enter_context(tc.tile_pool(name="expool", bufs=18))
        small = ctx.enter_context(tc.tile_pool(name="small", bufs=10))
        otp = ctx.enter_context(tc.tile_pool(name="otp", bufs=8))
        ocp_pool = ctx.enter_context(tc.tile_pool(name="ocp", bufs=3))
        xin = ctx.enter_context(tc.tile_pool(name="xin", bufs=13))
        wpool = ctx.enter_context(tc.tile_pool(name="wts", bufs=3))

        # ---- persistent SBUF tiles ----
        qTs = [persist.tile([128, T], bf16, tag=f"qT{p}", name=f"qT{p}") for p in range(NP)]
        kTs = [persist.tile([128, S], bf16, tag=f"kT{p}", name=f"kT{p}") for p in range(NP)]
        vts = [persist.tile([128, DC], bf16, tag=f"v{st}", name=f"v{st}") for st in range(NS)]
        WoSs = [persist.tile([128, E], bf16, tag=f"wo{p}", name=f"wo{p}") for p in range(NP)]
        Onorm = [persist.tile([128, T], bf16, tag=f"on{p}", name=f"on{p}") for p in range(NP)]
        bq_sb = persist.tile([128, NP], f32, tag="bq", name="bq_sb")
        bk_sb = persist.tile([128, NP], f32, tag="bk", name="bk_sb")
        bv_sb = persist.tile([128, DC], f32, tag="bv", name="bv_sb")
        ones_sb = persist.tile([128, 1], bf16, tag="ones", name="ones_sb")

        # biases are tiny and gate every projection drain — load them first
        nc.sync.dma_start(out=bq_sb, in_=bq_d[:, :])
        nc.sync.dma_start(out=bk_sb, in_=bk_d[:, :])
        bv_ap = bv_d[:, :]
        bv_bcast_ap = bass.AP(
            tensor=bv_ap.tensor,
            offset=bv_ap.offset,
            ap=[[0, 128], bv_ap.ap[-1]],
        )
        nc.sync.dma_start(out=bv_sb, in_=bv_bcast_ap)
        nc.vector.memset(ones_sb, 1.0)

        def load_late_inputs():
            """Output-projection weights: not needed until stage 12, so
            keep them off the startup DMA critical path."""
            for p in range(NP):
                nc.sync.dma_start(out=WoSs[p],
                                  in_=WoS_d[p * 128:(p + 1) * 128, :])

        def load_wall(dram):
            """All 8 e-chunks of one weight set, as two strided DMAs (so
            the first projection matmuls can start after half the bytes):
            wall[:, e*DC + c] = dram[e*128 + p, c]."""
            t_ = wpool.tile([128, 8 * DC], bf16, tag="w", name="wall")
            for g in range(2):
                nc.sync.dma_start(
                    out=t_[:, g * 4 * DC:(g + 1) * 4 * DC
                           ].rearrange("p (e c) -> p e c", c=DC),
                    in_=dram[g * 512:(g + 1) * 512, :
                             ].rearrange("(e p) c -> p e c", p=128))
            return t_

        def proj_thunks(p, x_dram, w_tiles, dst, bias_sb, halves=(0, 1)):
            """One pair's q/k projection as a thunk list: two column-halves;
            per half, the open thunk prefetches all 8 e-tile chunk DMAs
            (so the matmuls never wait on SP-queue DMA latency), then the
            two 512-col quarters run sequentially — each in its own single
            PSUM tile (8 accumulating MMs + a bias-add drain) so the two
            pmx bufs ping-pong between a quarter's MMs and the previous
            quarter's DVE drain."""
            thunks = []
            for half in halves:
                xh = []

                def open_half(half=half, xh=xh):
                    for j in range(4):
                        xt = xin.tile([128, 2048], bf16, tag="xin", name="xin")
                        nc.sync.dma_start(
                            out=xt.rearrange("p (e t) -> p e t", e=2),
                            in_=x_dram[j * 256:(j + 1) * 256,
                                       half * 1024:(half + 1) * 1024
                                       ].rearrange("(e p) t -> p e t", p=128))
                        xh.append(xt)

                thunks.append(open_half)

                for qi in range(2):
                    ps = []

                    def echunk(e, qi=qi, half=half, ps=ps, xh=xh):
                        if e == 0:
                            ps.append(pmx.tile([128, 512], f32, tag="mx",
                                               name="mx_ps"))
                        nc.tensor.matmul(
                            ps[0],
                            w_tiles[:, e * DC + p * 128:e * DC + (p + 1) * 128],
                            xh[e // 2][:, (e % 2) * 1024 + qi * 512:
                                       (e % 2) * 1024 + (qi + 1) * 512],
                            start=(e == 0),
                            stop=(e == 7),
                        )

                    def emms(qi=qi, lo=0, hi=4, f=echunk):
                        for e in range(lo, hi):
                            f(e)

                    thunks.append(lambda f=emms: f(lo=0, hi=4))
                    thunks.append(lambda f=emms: f(lo=4, hi=8))

                    def close_q(qi=qi, half=half, ps=ps, xh=xh):
                        q = half * 2 + qi
                        nc.vector.tensor_scalar_add(
                            dst[:, q * 512:(q + 1) * 512],
                            ps[0], bias_sb[:, p:p + 1])
                        ps.clear()
                        if qi == 1:
                            xh.clear()

                    thunks.append(close_q)
            return thunks

        def vproj_thunks(wv_tiles, dh):
            """V projection for head-quad dh (4 heads, N=256), streamed in
            two s-halves. dh=0 feeds pairs 0-1 (needed by stage 1); dh=1
            feeds pairs 2-3 (needed from stage 9) and can spread late."""
            thunks = []
            for half in range(2):
                vh = []

                def load_half(half=half, vh=vh):
                    for j in range(4):
                        vt = xin.tile([128, 2048], bf16, tag="xin", name="xin")
                        nc.sync.dma_start(
                            out=vt.rearrange("p (e t) -> p e t", e=2),
                            in_=vT_d[j * 256:(j + 1) * 256,
                                     half * 1024:(half + 1) * 1024
                                     ].rearrange("(e p) t -> p e t", p=128))
                        vh.append(vt)

                thunks.append(load_half)
                for sti in range(8):
                    def vst(sti=sti, half=half, vh=vh):
                        st = half * 8 + sti
                        ps = pmx.tile([128, 512], f32, tag="mx", name="mx_ps")
                        for e in range(8):
                            nc.tensor.matmul(
                                ps[:, 0:256],
                                vh[e // 2][:, (e % 2) * 1024 + sti * 128:
                                           (e % 2) * 1024 + (sti + 1) * 128],
                                wv_tiles[:, e * DC + dh * 256:
                                         e * DC + (dh + 1) * 256],
                                start=(e == 0),
                                stop=(e == 7),
                            )
                        nc.vector.tensor_add(
                            vts[st][:, dh * 256:(dh + 1) * 256],
                            ps[:, 0:256],
                            bv_sb[:, dh * 256:(dh + 1) * 256],
                        )
                        if half == 0 and sti == 7:
                            vh.clear()
                    thunks.append(vst)
            return thunks

        def outproj_thunks(tq):
            thunks = []
            for tt in range(tq * 4, tq * 4 + 4):
                for c in range(2):
                    def unit(tt=tt, c=c):
                        op_ps = pmx.tile([128, 512], f32, tag="mx", name="mx_ps")
                        for p in range(NP):
                            nc.tensor.matmul(
                                op_ps,
                                Onorm[p][:, tt * 128:(tt + 1) * 128],
                                WoSs[p][:, c * 512:(c + 1) * 512],
                                start=(p == 0),
                                stop=(p == 3),
                            )
                        oc = ocp_pool.tile([128, 512], f32, tag="ocp", name="oc")
                        nc.vector.tensor_copy(oc, op_ps)
                        nc.sync.dma_start(
                            out=out_d[tt * 128:(tt + 1) * 128,
                                      c * 512:(c + 1) * 512],
                            in_=oc)
                    thunks.append(unit)
            return thunks

        class PrevStage:
            def __init__(self, p, tq, exs):
                self.p, self.tq, self.exs = p, tq, exs
                self.acc = None   # [128 t, 512]: 8 x 64-wide accums, idx 4h+k
                self.den = None   # [128 t, 16]: cols 2k+h

        def emit_pv_slot(prev, st):
            """PV for one s-tile of the previous stage: per (head h, t-tile
            k), a 64-wide main matmul (exp stationary, v moving) plus a
            1-wide denominator matmul against the ones column. One PSUM
            accumulation group per bank (lazy per-byte init)."""
            for h in range(2):
                vsl = vts[st][:, (2 * prev.p + h) * 64:(2 * prev.p + h + 1) * 64]
                for k in range(4):
                    idx = 4 * h + k
                    exsl = prev.exs[st][:, h * 512 + k * 128:
                                        h * 512 + (k + 1) * 128]
                    first = (st == 0 and idx == 0)
                    last = (st == NS - 1 and idx == 7)
                    nc.tensor.matmul(
                        prev.acc[:, idx * 64:(idx + 1) * 64],
                        exsl, vsl,
                        start=first, stop=last,
                    )
                    nc.tensor.matmul(
                        prev.den[:, 2 * k + h:2 * k + h + 1],
                        exsl, ones_sb,
                        start=first, stop=last,
                    )

        def emit_normalize(prev):
            """Normalize + transpose the previous stage's PV accumulators:
            one reciprocal over all 8 denominators, then per t-tile k:
            scalar-multiply both heads' 64 cols into a [t, 128] bf16 tile
            and DMA-transpose (SBUF->SBUF) into the [d, t]-major Onorm
            slice."""
            t0 = prev.tq * 4
            rc = small.tile([128, 8], f32, tag="rc", name="rc")
            nc.vector.reciprocal(rc, prev.den[:, 0:8])
            for k in range(4):
                ot = otp.tile([128, 128], bf16, tag="ot", name="ot")
                for h in range(2):
                    nc.vector.tensor_scalar_mul(
                        ot[:, h * 64:(h + 1) * 64],
                        prev.acc[:, (4 * h + k) * 64:(4 * h + k + 1) * 64],
                        rc[:, 2 * k + h:2 * k + h + 1],
                    )
                nc.sync.dma_start(
                    out=Onorm[prev.p][:, (t0 + k) * 128:(t0 + k + 1) * 128],
                    in_=ot, transpose=True)

        def emit_stage(p, tq, prev, extras, dl=6, pv_start=0):
            """16 score slots for (p, tq); interleave prev stage's PV at two
            PV-slots per score slot from `pv_start` (all its exp tiles
            already exist), normalizing mid-stage so the single PV
            accumulator bank clears well before the next stage reuses it;
            extra thunks all emitted by slot `dl`. Returns this stage's
            PrevStage record."""
            t0 = tq * 512
            exs = []
            if prev is not None:
                prev.acc = pacc.tile([128, 512], f32, tag="acc", name="acc_ps")
                prev.den = pden.tile([128, 16], f32, tag="den", name="den_ps")
            n_ex = len(extras)
            taken = 0
            pv_done = 0
            for st in range(NS):
                sc_ps = psc.tile([128, 1024], f32, tag="sc", name="sc_ps")
                nc.tensor.matmul(
                    sc_ps[:, 0:512],
                    kTs[p][0:64, st * 128:(st + 1) * 128],
                    qTs[p][0:64, t0:t0 + 512],
                    start=True, stop=True,
                    tile_position=(0, 0),
                )
                nc.tensor.matmul(
                    sc_ps[:, 512:1024],
                    kTs[p][64:128, st * 128:(st + 1) * 128],
                    qTs[p][64:128, t0:t0 + 512],
                    start=True, stop=True,
                    tile_position=(64, 0),
                )
                ex = expool.tile([128, 1024], bf16, tag="ex", name="ex")
                nc.scalar.activation(ex, sc_ps, AF.Exp, scale=0.125)
                exs.append(ex)
                if prev is not None and st >= pv_start:
                    for _ in range(2):
                        if pv_done < NS:
                            emit_pv_slot(prev, pv_done)
                            pv_done += 1
                    if pv_done == NS:
                        emit_normalize(prev)
                        pv_done = NS + 1
                want = (n_ex * min(st + 1, dl) + dl - 1) // dl
                while taken < want:
                    extras[taken]()
                    taken += 1
            while taken < n_ex:
                extras[taken]()
                taken += 1
            if prev is not None and pv_done <= NS:
                while pv_done < NS:
                    emit_pv_slot(prev, pv_done)
                    pv_done += 1
                emit_normalize(prev)
            return PrevStage(p, tq, exs)

        # ---- emission ----
        for _rep in range(repeats):
            # startup: only the FIRST quarters of pair-0's q/k projections
            # block the first four score slots; everything else overlaps.
            wq_tiles = load_wall(WqT_d)
            q0h0 = proj_thunks(0, qT_d, wq_tiles, qTs[0], bq_sb, halves=(0,))
            wk_tiles = load_wall(WkT_d)
            k0h0 = proj_thunks(0, kT_d, wk_tiles, kTs[0], bk_sb, halves=(0,))
            q0 = proj_thunks(0, qT_d, wq_tiles, qTs[0], bq_sb, halves=(1,))
            k0 = proj_thunks(0, kT_d, wk_tiles, kTs[0], bk_sb, halves=(1,))
            # issue all four halves' preload DMAs before any matmul runs;
            # v weights + the first vT load-half jump ahead of the WoS
            # loads (needed ~200us later) in the DMA queue
            q0h0[0](); k0h0[0](); k0[0](); q0[0]()
            wv_tiles = load_wall(WvT_d)
            vpA = vproj_thunks(wv_tiles, 0)
            vpA[0](); vpA[9]()     # both vT load-halves ahead of WoS
            load_late_inputs()
            for th in q0h0[1:]:
                th()
            for th in k0h0[1:]:
                th()

            # per-stage extra work, placed just-in-time:
            #  stage 0: K0/Q0 second halves + V-projection first s-half
            #  stage 1: V-projection second s-half
            #  pair p>=1: opens a stage ahead; K-half0 @4p-2, Q-half0 @4p-1,
            #           K-half1 @4p, Q-half1 @4p+1
            #  stages 14, 15: out-proj for t0, t1
            extras = {0: k0[1:] + q0[1:] + vpA[1:9], 1: vpA[10:]}
            for p in range(1, NP):
                qp = proj_thunks(p, qT_d, wq_tiles, qTs[p], bq_sb)
                kp = proj_thunks(p, kT_d, wk_tiles, kTs[p], bk_sb)
                # each half's 8-DMA open goes a full stage before its MMs so
                # the matmuls never wait on SP-queue DMA delivery
                for sg, th in ((4 * p - 3, kp[0:1] + qp[0:1]),
                               (4 * p - 2, kp[1:7]),
                               (4 * p - 1, qp[1:7] + kp[7:8]),
                               (4 * p, kp[8:14] + qp[7:8]),
                               (4 * p + 1, qp[8:14])):
                    extras[sg] = extras.get(sg, []) + th
            # second head-quad of V, appended after each stage's proj work
            # (sequential mx-slot handoff, done well before stage 9 needs
            # it). The two load-halves go a stage ahead of their matmuls so
            # the vproj MMs never wait on the SP DMA queue. Skip stages 4/5
            # whose proj work has a hard slot-8 deadline (dl=7).
            vpB = vproj_thunks(wv_tiles, 1)
            extras[1] = extras.get(1, []) + vpB[0:1]       # load half 0
            extras[2] = extras.get(2, []) + vpB[1:5]
            extras[3] = extras.get(3, []) + vpB[5:10]      # incl. load half 1
            extras[6] = extras.get(6, []) + vpB[10:13]
            extras[7] = extras.get(7, []) + vpB[13:18]
            # defer the out-proj units ~3 slots so the pair-3 Onorm
            # DMA-transposes (mid-previous-stage) have certainly landed
            nops = [lambda: None] * 3
            extras[14] = extras.get(14, []) + nops + outproj_thunks(0)
            extras[15] = extras.get(15, []) + nops + outproj_thunks(1)

            # pacing deadlines: K-half1 stages (4p) must finish extras by
            # slot 8 (their own scores need those kT columns); stages 0/1
            # feed vts just-in-time (stage 1's PV starts late so the last
            # vts s-tiles land first); elsewhere spread smoothly.
            dls = {0: 14, 1: 8, 4: 7, 8: 7, 12: 7}
            pv_starts = {1: 5}
            prev = None
            for s in range(16):
                p, tq = s // 4, s % 4
                prev = emit_stage(p, tq, prev, extras.get(s, []),
                                  dl=dls.get(s, 16),
                                  pv_start=pv_starts.get(s, 1))

            # tail: PV of the last stage with out-proj(t2) interleaved
            # (its Onorm slices completed at the end of stage 15), then the
            # final normalize and out-proj(t3)
            prev.acc = pacc.tile([128, 512], f32, tag="acc", name="acc_ps")
            prev.den = pden.tile([128, 16], f32, tag="den", name="den_ps")
            op2 = outproj_thunks(2)
            for st in range(NS):
                emit_pv_slot(prev, st)
                if st % 2 == 1:
                    op2[st // 2]()
            # final normalize interleaved per t-tile with its wide out-proj
            # unit, so PE streams op-t3 matmuls while DVE/DMA finish the
            # next tile's normalize + transpose
            # all four transposes dispatch before the first op unit's
            # out-DMA can queue behind them on SP
            rc = small.tile([128, 8], f32, tag="rc", name="rc")
            nc.vector.reciprocal(rc, prev.den[:, 0:8])
            for k in range(4):
                ot = otp.tile([128, 128], bf16, tag="ot", name="ot")
                for h in range(2):
                    nc.vector.tensor_scalar_mul(
                        ot[:, h * 64:(h + 1) * 64],
                        prev.acc[:, (4 * h + k) * 64:(4 * h + k + 1) * 64],
                        rc[:, 2 * k + h:2 * k + h + 1],
                    )
                nc.sync.dma_start(
                    out=Onorm[3][:, (12 + k) * 128:(13 + k) * 128],
                    in_=ot, transpose=True)
            for k in range(4):
                tt = 12 + k
                op_ps = psc.tile([128, 1024], f32, tag="sc", name="sc_ps")
                for c in range(2):
                    for p in range(NP):
                        nc.tensor.matmul(
                            op_ps[:, c * 512:(c + 1) * 512],
                            Onorm[p][:, tt * 128:(tt + 1) * 128],
                            WoSs[p][:, c * 512:(c + 1) * 512],
                            start=(p == 0),
                            stop=(p == 3),
                        )
                oc = ocp_pool.tile([128, 1024], f32, tag="ocpw", name="ocw")
                nc.vector.tensor_copy(oc, op_ps)
                nc.sync.dma_start(out=out_d[tt * 128:(tt + 1) * 128, :], in_=oc)

    nc.compile()
    return nc


def _get_nc():
    global _cached
    if _cached is None:
        _cached = _build()
    return _cached


def _prep_core_inputs(c, query, key, value, Wq, Wk, Wv, Wo, bq, bk, bv,
                      _cache={}):
    b, g = c // 2, c % 2
    sl = slice(g * DC, (g + 1) * DC)
    key_ = (id(query), b)
    if key_ not in _cache:
        # both cores of a batch share the transposed/cast activations
        _cache.clear()
        _cache[key_] = {
            "qT": query[b].T.astype(_BF16),
            "kT": key[b].T.astype(_BF16),
            "vT": value[b].T.astype(_BF16),
        }
    shared = _cache[key_]
    return {
        **shared,
        "WqT": Wq[sl].T.astype(_BF16),
        "WkT": Wk[sl].T.astype(_BF16),
        "WvT": Wv[sl].T.astype(_BF16),
        "WoS": Wo[:, sl].T.astype(_BF16),
        "bq": np.ascontiguousarray(bq[sl].reshape(NP, 128).T),
        "bk": np.ascontiguousarray(bk[sl].reshape(NP, 128).T),
        "bv": np.ascontiguousarray(bv[sl].reshape(1, DC)),
    }


def kernel(**inputs):
    from concourse.bass_utils import run_bass_kernel_spmd

    args = {k: np.asarray(inputs[k], np.float32)
            for k in ("query", "key", "value", "Wq", "Wk", "Wv", "Wo",
                      "bq", "bk", "bv", "bo")}
    _prep_core_inputs.__defaults__[0].clear()
    nc = _get_nc()
    in_maps = [
        _prep_core_inputs(c, args["query"], args["key"], args["value"],
                          args["Wq"], args["Wk"], args["Wv"], args["Wo"],
                          args["bq"], args["bk"], args["bv"])
        for c in range(8)
    ]
    res = run_bass_kernel_spmd(nc, in_maps, core_ids=list(range(8)))
    outs = [r["out"] for r in res.results]
    final = np.empty((B, T, E), np.float32)
    for b in range(B):
        final[b] = outs[2 * b] + outs[2 * b + 1] + args["bo"][None, :]
    return final
